# revision 25
# baseline (speedup 1.0000x reference)
"""Exact KNN collision kernel for trn2 (8 NeuronCores).

Computes nn[b,n] = argmin_m |vertices[b,n] - collider[b, cvi[m]]|^2 with the
reference's exact fp32 arithmetic and first-occurrence tie-breaking.

Strategy per core (core c -> batch b=c//2, row-half h=c%2, 8192 rows):
  - host dedups gathered collider points; candidates stored in REVERSED
    dedup-slot order (col k = slot U-1-k)
  - PE: dot = v @ cv^T  (K=3 fp32 matmul, bitwise equal to the reference
    einsum on this backend; 128-row x 512-col chunks -> PSUM). PSUM is
    carved into four independently-tagged 1024-col pieces so the ACT
    drain of tile t never stalls the PE matmuls of tile t+1.
  - ACT: copy each PSUM piece -> SBUF as soon as the PE fills it
  - DVE: ONE fused custom op per row tile (replaces the old sub+max pass
    and the max_index pass):
        s = dot - c2/2; running scan-max; accum = last index where s equals
        the running max  == last occurrence of the row max in stream order
        == smallest dedup slot among exact ties (reversed layout)
    This matches the reference argmin tie-break (first occurrence) exactly
    because s == -d2/2 bitwise.
  - host maps j* -> slot U-1-j* -> first position in collision_vertices

Perf: PE-bound at fp32's 4 cycles/column (exactness requires the fp32
matmul path: f32r and bf16-split matmuls are not bitwise equal to the
reference and flip ~50 exact-tie rows). 465us baseline -> 365us.
"""
import os
import sys
import numpy as np

_BASS_PATH = "/opt/trn_rl_repo"
if _BASS_PATH not in sys.path:
    sys.path.insert(0, _BASS_PATH)

B, N, V, M = 4, 16384, 6890, 4096
NCORES = 8
ROWS = (B * N) // NCORES          # 8192 rows per core
NT = ROWS // 128                  # 64 row tiles
VARIANT = os.environ.get("KNN_VARIANT", "scan")
NEG = np.float32(-3.4028235e38)

_PROGRAM_CACHE = {}


def _register_op(name, make_spec):
    from concourse import dve_ops
    from concourse.dve_spec import lower
    from concourse.dve_spec import _has_src1
    from concourse.dve_uop import DveOpSpec

    if name in dve_ops._SUB_OPCODE_FOR_NAME:
        return dve_ops.CUSTOM_DVE_SPECS[name]._antop
    spec = make_spec()
    shas = {}
    for ver in ("v3", "v4"):
        tmp = DveOpSpec(name=name, opcode=31, uops=lower(spec, ver=ver),
                        rd1_en=_has_src1(spec))
        shas[ver] = tmp.sha(ver)
    op = dve_ops.DveOp(name, spec, subdim=False, uops_sha=shas)
    row = max(dve_ops._SUB_OPCODE_FOR_NAME.values()) + 1
    assert row < 0x20
    dve_ops.OPS.append(op)
    dve_ops.CUSTOM_DVE_SPECS[name] = spec
    dve_ops._SUB_OPCODE_FOR_NAME[name] = row
    spec._antop = op
    return op


def _register_sub_argmax_scan():
    """out = s = in0-in1 (masked select stream, dead); accum_out = index of
    the LAST element equal to the running max of s (fp32 index)."""
    from concourse.dve_spec import (Spec, Src0, Src1, Idx, MaxNeg, maxx,
                                    select, scan, AluOp)

    def make():
        def _ref(in0, in1, c0, c1, c2):
            s = (np.asarray(in0, np.float32)
                 - np.asarray(in1, np.float32)).astype(np.float32)
            s2 = s.reshape(s.shape[0], -1)
            m = np.maximum.accumulate(s2, axis=-1)
            idx = np.broadcast_to(
                np.arange(s2.shape[1], dtype=np.float32), s2.shape)
            body = np.where(s2 >= m, idx, NEG).astype(np.float32)
            acc = body.max(axis=-1, keepdims=True).astype(np.float32)
            return body.reshape(s.shape), acc

        s = Src0 - Src1
        sm = scan(AluOp.MAX, s)
        body = select(s >= sm, Idx, MaxNeg)
        return Spec(body=body, accum=maxx, reference=_ref)

    return _register_op("SUB_ARGMAX_SCAN_ANT", make)


def _register_sub_max():
    """Baseline variant op: out = in0 - in1; accum = max (row max)."""
    from concourse.dve_spec import Spec, Src0, Src1, C0, maxx

    def make():
        def _ref(in0, in1, c0, c1, c2):
            body = (np.asarray(in0, np.float32)
                    - np.asarray(in1, np.float32)).astype(np.float32)
            seed = np.asarray(c0, np.float32).reshape(-1, 1)
            acc = np.maximum(np.maximum.reduce(
                body.reshape(body.shape[0], -1), axis=-1, keepdims=True), seed)
            return body, acc
        return Spec(body=Src0 - Src1, accum=maxx, accum_init=C0, reference=_ref)

    return _register_op("SUB_MAX_REDUCE_ANT", make)


def _build_program(U, rows=ROWS, nt=NT):
    import concourse.bacc as bacc
    import concourse.mybir as mybir
    import concourse.tile as tile

    f32 = mybir.dt.float32
    u32 = mybir.dt.uint32
    MP = ((U + 511) // 512) * 512

    nc = bacc.Bacc("TRN2", target_bir_lowering=False, debug=False, num_devices=NCORES)
    vc = nc.dram_tensor("vc", [3, rows + MP], f32, kind="ExternalInput")
    c2h = nc.dram_tensor("c2h", [1, MP], f32, kind="ExternalInput")
    out = nc.dram_tensor("idx", [128, nt], f32, kind="ExternalOutput")

    # matmul chunk column ranges covering exactly [0, U)
    chunks = []
    j0 = 0
    while j0 < U:
        chunks.append((j0, min(j0 + 512, U)))
        j0 += 512

    with tile.TileContext(nc) as tc:
        with (
            tc.tile_pool(name="const", bufs=1) as cpool,
            tc.tile_pool(name="work", bufs=2) as wpool,
            tc.tile_pool(name="psum", bufs=1, space="PSUM") as ppool,
        ):
            cv_sb = cpool.tile([3, MP], f32)
            c2h_sb = cpool.tile([128, MP], f32)
            c2row = cpool.tile([1, MP], f32)
            ones = cpool.tile([1, 128], f32)
            nt0 = max(nt // 2, 1)
            acc0 = cpool.tile([128, nt0], f32)
            acc1 = cpool.tile([128, max(nt - nt0, 1)], f32)
            # vertex columns in separate tiles, DMA'd individually: the
            # first row tiles only wait for candidates + their own block,
            # not the whole (per-partition-narrow, slow) input transfer
            NVB = 8 if nt % 8 == 0 else 1
            vblk = rows // NVB
            vts = [cpool.tile([3, vblk], f32, tag=f"v{i}", name=f"v{i}")
                   for i in range(NVB)]
            nc.sync.dma_start(c2row[:], c2h[:])
            nc.sync.dma_start(cv_sb[:], vc[:, rows:rows + MP])
            for i in range(NVB):
                nc.sync.dma_start(vts[i][:], vc[:, i * vblk:(i + 1) * vblk])
            nc.gpsimd.memset(ones[:], 1.0)

            subop = _register_sub_argmax_scan()

            # PSUM in 4 independently-tagged 1024-col pieces: ACT drains each
            # piece as soon as the PE fills it, so PE(t+1) never stalls on
            # the drain of tile t (tile-granular dependency tracking).
            PIECE = 1024
            pieces = []
            p0 = 0
            while p0 < MP:
                pieces.append((p0, min(p0 + PIECE, MP)))
                p0 += PIECE

            def dot_tiles(tag_prefix):
                return [ppool.tile([128, b - a], f32, tag=f"{tag_prefix}{i}",
                                   name=f"{tag_prefix}{i}")
                        for i, (a, b) in enumerate(pieces)]

            def emit_mm(dst_tiles, lhs, rhs_fn):
                for (a, b) in chunks:
                    pi = a // PIECE
                    pa, _ = pieces[pi]
                    nc.tensor.matmul(
                        dst_tiles[pi][:, a - pa:b - pa],
                        lhs, rhs_fn(a, b),
                        start=True, stop=True,
                    )

            def emit_drain(dst, dst_off, src_tiles):
                for pi, (a, b) in enumerate(pieces):
                    hi = min(b, U)
                    if hi <= a:
                        continue
                    nc.scalar.copy(dst[:, dst_off + a:dst_off + hi],
                                   src_tiles[pi][:, :hi - a])

            # replicate c2row across 128 partitions: ones^T @ c2row via the PE,
            # staged through the dot PSUM pieces, copied out by the ACT engine
            rep = dot_tiles("dot")
            emit_mm(rep, ones[:], lambda a, b: c2row[:, a:b])
            emit_drain(c2h_sb, 0, rep)

            tpb = vblk // 128           # row tiles per vertex block
            nt0 = max(nt // 2, 1)       # tiles in the first output half
            for t in range(nt):
                dott = dot_tiles("dot")
                vb = vts[t // tpb]
                o = (t % tpb) * 128
                emit_mm(dott, vb[:, o:o + 128],
                        lambda a, b: cv_sb[:, a:b])
                dcp = wpool.tile([128, U], f32, tag="dcp")
                emit_drain(dcp, 0, dott)
                scr = wpool.tile([128, U], f32, tag="scr")
                acct = (acc0[:, t:t + 1] if t < nt0
                        else acc1[:, t - nt0:t - nt0 + 1])
                nc.vector._custom_dve(
                    subop, out=scr[:], in0=dcp[:], in1=c2h_sb[:, :U],
                    accum_out=acct)
                if t == nt0 - 1:
                    # first half of the results leaves early, overlapping
                    # the remaining tiles instead of the program tail
                    nc.sync.dma_start(out[:, :nt0], acc0[:])
            nc.sync.dma_start(out[:, nt0:], acc1[:])
    nc.compile()
    return nc


def _get_program(U, rows=ROWS, nt=NT):
    key = ("exact", U, rows, nt)
    if key not in _PROGRAM_CACHE:
        _PROGRAM_CACHE[key] = _build_program(U, rows, nt)
    return _PROGRAM_CACHE[key]


def _build_program_p1(UP):
    """Round-1 (noisy, fast): bf16-split K=30 matmul -> s' in PSUM; GPSIMD
    pairwise-max tree (groups of 4 consecutive slots); DVE max8 + max_index
    -> top-8 group values + indices per row."""
    import concourse.bacc as bacc
    import concourse.mybir as mybir
    import concourse.tile as tile

    f32 = mybir.dt.float32
    u32 = mybir.dt.uint32
    bf16 = mybir.dt.bfloat16
    assert UP % 8 == 0 and 2048 < UP <= 4096
    NB = UP - 2048
    H = UP // 2
    T = UP // 4

    nc = bacc.Bacc("TRN2", target_bir_lowering=False, debug=False, num_devices=NCORES)
    vc = nc.dram_tensor("vc16", [30, ROWS + UP], bf16, kind="ExternalInput")
    i8d = nc.dram_tensor("idx8", [NT, 128, 8], u32, kind="ExternalOutput")
    v8d = nc.dram_tensor("val8", [NT, 128, 8], f32, kind="ExternalOutput")

    chunks = []
    j0 = 0
    while j0 < UP:
        chunks.append((j0, min(j0 + 512, UP)))
        j0 += 512

    with tile.TileContext(nc) as tc:
        with (
            tc.tile_pool(name="const", bufs=1) as cpool,
            tc.tile_pool(name="work", bufs=2) as wpool,
            tc.tile_pool(name="psum", bufs=1, space="PSUM") as ppool,
        ):
            vc_sb = cpool.tile([30, ROWS + UP], bf16)
            nc.sync.dma_start(vc_sb[:], vc[:])

            for t in range(NT):
                dotA = ppool.tile([128, 2048], f32, tag="dotA")
                dotB = ppool.tile([128, NB], f32, tag="dotB")
                for (a, b) in chunks:
                    dst = dotA[:, a:b] if b <= 2048 else dotB[:, a - 2048:b - 2048]
                    nc.tensor.matmul(
                        dst,
                        vc_sb[:, t * 128:(t + 1) * 128],
                        vc_sb[:, ROWS + a:ROWS + b],
                        start=True, stop=True,
                    )
                s = wpool.tile([128, UP], f32, tag="s")
                m = wpool.tile([128, H], f32, tag="m")
                mm = wpool.tile([128, T], f32, tag="mm")
                v8 = wpool.tile([128, 8], f32, tag="v8")
                i8 = wpool.tile([128, 8], u32, tag="i8")
                # GPSIMD cannot read PSUM: ACT drains it to SBUF first.
                # Contiguous-halves pairing (Pool rejects strided TT):
                # group j = slots {j, j+T, j+2T, j+3T}
                nc.scalar.copy(s[:, 0:2048], dotA[:])
                nc.scalar.copy(s[:, 2048:UP], dotB[:])
                nc.gpsimd.tensor_max(m[:], s[:, 0:H], s[:, H:UP])
                nc.gpsimd.tensor_max(mm[:], m[:, 0:T], m[:, T:H])
                nc.vector.max(v8[:], mm[:])
                nc.vector.max_index(i8[:], v8[:], mm[:])
                nc.sync.dma_start(i8d[t], i8[:])
                nc.sync.dma_start(v8d[t], v8[:])
    nc.compile()
    return nc


def _get_program_p1(UP):
    key = ("p1", UP)
    if key not in _PROGRAM_CACHE:
        _PROGRAM_CACHE[key] = _build_program_p1(UP)
    return _PROGRAM_CACHE[key]


def _trunc16(x):
    return (np.ascontiguousarray(x, np.float32).view(np.uint32)
            & np.uint32(0xFFFF0000)).view(np.float32)


def _split3(x):
    a = _trunc16(x)
    r = (x - a).astype(np.float32)
    b = _trunc16(r)
    cc = (r - b).astype(np.float32)
    return a, b, cc


def _trunc12(x):
    return (np.ascontiguousarray(x, np.float32).view(np.uint32)
            & np.uint32(0xFFFFF000)).view(np.float32)


def _emul_dot(vrow, cand):
    """Bitwise-faithful (to ~1 ulp) emulation of the device fp32 K=3 matmul:
    weight (vertex) split at 11+1 bits; two exactly-rounded passes; fp32 add.
    vrow [R,3] f32, cand [R,Q,3] f32 -> dot [R,Q] f32."""
    vh = _trunc12(vrow)
    vl = (vrow - vh).astype(np.float32)
    p1 = np.zeros(cand.shape[:2], np.float64)
    p2 = np.zeros(cand.shape[:2], np.float64)
    for d in range(3):
        cd = cand[:, :, d].astype(np.float64)
        p1 += vh[:, d:d + 1].astype(np.float64) * cd
        p2 += vl[:, d:d + 1].astype(np.float64) * cd
    return (p1.astype(np.float32).astype(np.float64)
            + p2.astype(np.float32).astype(np.float64)).astype(np.float32)


# term order for the K=30 split matmul: ascending magnitude
_TERMS = [(2, 2), (1, 2), (2, 1), (0, 2), (2, 0), (1, 1), "h2",
          (0, 1), (1, 0), "h1", (0, 0), "h0"]


def _build_split_rows(vparts, cparts, hparts, ones_len):
    """Build the [30, *] lhs/rhs row stacks for the bf16-split matmul.
    vparts/cparts: [3 parts][L, 3]; hparts: [3 parts][Lc]."""
    lhs_rows = []
    rhs_rows = []
    for t in _TERMS:
        if isinstance(t, str):
            k = int(t[1])
            lhs_rows.append(np.ones(ones_len, np.float32))
            rhs_rows.append(-hparts[k])
        else:
            i, j = t
            for d in range(3):
                lhs_rows.append(vparts[i][:, d])
                rhs_rows.append(cparts[j][:, d])
    return np.stack(lhs_rows), np.stack(rhs_rows)


def kernel(vertices, collider, collision_vertices, _want_trace=False):
    from concourse.bass_utils import run_bass_kernel_spmd

    v = np.ascontiguousarray(np.asarray(vertices), dtype=np.float32)     # [B,N,3]
    c = np.ascontiguousarray(np.asarray(collider), dtype=np.float32)     # [B,V,3]
    cvi = np.asarray(collision_vertices).astype(np.int64)                # [M]

    # dedup candidates, keeping first-occurrence order (exact tie semantics)
    u, first_pos = np.unique(cvi, return_index=True)
    order = np.argsort(first_pos)
    u = u[order]
    first_pos = first_pos[order].astype(np.int32)

    if VARIANT == "fast2":
        return _kernel_fast2(v, c, u, first_pos, _want_trace)
    return _kernel_scan(v, c, u, first_pos, _want_trace)


def _pack_c2h_quarters(c2h_pad):
    q = np.full((4, 1024), np.float32(5e29), np.float32)
    mp = len(c2h_pad)
    for p in range(4):
        lo = p * 1024
        hi = min(lo + 1024, mp)
        if hi > lo:
            q[p, :hi - lo] = c2h_pad[lo:hi]
    return np.ascontiguousarray(q)


def _kernel_scan(v, c, u, first_pos, _want_trace):
    from concourse.bass_utils import run_bass_kernel_spmd
    U = len(u)
    MP = ((U + 511) // 512) * 512

    # REVERSED slot order: column k holds dedup slot U-1-k
    u_rev = u[::-1]
    cv = c[:, u_rev, :]                                          # [B,U,3]
    c2h = (cv * cv).sum(-1, dtype=np.float32) * np.float32(0.5)  # [B,U]

    cvT_pad = np.zeros((B, 3, MP), np.float32)
    cvT_pad[:, :, :U] = cv.transpose(0, 2, 1)
    c2h_pad = np.full((B, MP), np.float32(5e29), np.float32)
    c2h_pad[:, :U] = c2h

    in_maps = []
    for core in range(NCORES):
        b = core // 2
        r0 = (core % 2) * ROWS
        vT = v[b, r0:r0 + ROWS, :].T                             # [3, ROWS]
        in_maps.append({
            "vc": np.ascontiguousarray(
                np.concatenate([vT, cvT_pad[b]], axis=1), dtype=np.float32),
            "c2h": np.ascontiguousarray(c2h_pad[b][None, :], dtype=np.float32),
        })

    nc = _get_program(U)
    res = run_bass_kernel_spmd(nc, in_maps, core_ids=list(range(NCORES)))

    nn = np.zeros((B, N), np.int32)
    for core in range(NCORES):
        b = core // 2
        r0 = (core % 2) * ROWS
        j = res.results[core]["idx"]                 # [128, NT] f32 stream idx
        j = np.rint(j).astype(np.int64)
        slot = (U - 1) - j                           # back to dedup slot space
        nn_core = first_pos[slot.T.reshape(-1)]      # row (t*128+r) <- acc[r,t]
        nn[b, r0:r0 + ROWS] = nn_core
    batch_idx = np.broadcast_to(np.arange(B, dtype=np.int32)[:, None], nn.shape)
    outv = np.stack([batch_idx, nn], axis=-1).astype(np.int32)
    if _want_trace:
        return outv, (res, in_maps)
    return outv


def _kernel_fast2(v, c, u, first_pos, _want_trace):
    from concourse.bass_utils import run_bass_kernel_spmd
    import ml_dtypes

    U = len(u)
    UP = ((U + 7) // 8) * 8
    MP = ((U + 511) // 512) * 512

    cv = c[:, u, :]                                   # [B,U,3] plain dedup order
    xx, yy, zz = cv[..., 0], cv[..., 1], cv[..., 2]
    c2 = ((xx * xx + yy * yy) + zz * zz).astype(np.float32)   # == device c2 bitwise
    c2h = (c2 * np.float32(0.5)).astype(np.float32)

    cvp = np.zeros((B, UP, 3), np.float32)
    cvp[:, :U] = cv
    c2p = np.zeros((B, UP), np.float32)
    c2p[:, :U] = c2
    c2hp = np.full((B, UP), np.float32(5e29), np.float32)
    c2hp[:, :U] = c2h

    in_maps = []
    for core in range(NCORES):
        b = core // 2
        r0 = (core % 2) * ROWS
        vparts = _split3(v[b, r0:r0 + ROWS])
        cparts = _split3(cvp[b])
        hparts = _split3(c2hp[b])
        lhs, rhs = _build_split_rows(vparts, cparts, hparts, ROWS)
        vc16 = np.ascontiguousarray(
            np.concatenate([lhs, rhs], axis=1)).astype(ml_dtypes.bfloat16)
        in_maps.append({"vc16": vc16})

    nc1 = _get_program_p1(UP)
    res1 = run_bass_kernel_spmd(nc1, in_maps, core_ids=list(range(NCORES)))

    nn = np.zeros((B, N), np.int32)
    flag_rows = []
    for core in range(NCORES):
        b = core // 2
        r0 = (core % 2) * ROWS
        i8 = res1.results[core]["idx8"].reshape(ROWS, 8)
        v8 = res1.results[core]["val8"].reshape(ROWS, 8).astype(np.float32)
        g3 = i8[:, :3].astype(np.int64)
        T4 = UP // 4
        offs = np.array([0, T4, 2 * T4, 3 * T4], dtype=np.int64)
        slots = (g3[:, :, None] + offs).reshape(ROWS, 12)
        valid = slots < U
        sl = np.minimum(slots, U - 1)
        dot = _emul_dot(v[b, r0:r0 + ROWS], cvp[b][sl])
        d2 = (c2[b][sl] - np.float32(2.0) * dot).astype(np.float32)
        d2 = np.where(valid, d2, np.float32(np.inf))
        d2min = d2.min(1)
        ismin = d2 == d2min[:, None]
        wslot = np.where(ismin, sl, U).min(1)
        d2b = np.where(sl == wslot[:, None], np.float32(np.inf), d2)
        margin = d2b.min(1) - d2min
        derr = (np.abs(np.float32(2.0) * dot).max(1)
                * np.float32(6 * 2.0 ** -23) + np.float32(2e-7))
        vgap = v8[:, 0] - v8[:, 3]
        flg = (margin <= derr) | (vgap < np.float32(2e-5))
        nn[b, r0:r0 + ROWS] = first_pos[np.minimum(wslot, U - 1)]
        flag_rows.append(np.nonzero(flg)[0])

    ATm = max((len(f) + 127) // 128 for f in flag_rows)
    res2 = None
    in2 = None
    if ATm > 0:
        rows2 = ATm * 128
        cvr = cv[:, ::-1, :]
        c2hr = c2h[:, ::-1]
        cvT_pad = np.zeros((B, 3, MP), np.float32)
        cvT_pad[:, :, :U] = cvr.transpose(0, 2, 1)
        c2h_pad = np.full((B, MP), np.float32(5e29), np.float32)
        c2h_pad[:, :U] = c2hr
        in2 = []
        for core in range(NCORES):
            b = core // 2
            r0 = (core % 2) * ROWS
            fr = flag_rows[core]
            vg = np.zeros((rows2, 3), np.float32)
            if len(fr):
                vg[:len(fr)] = v[b, r0 + fr]
            in2.append({
                "vc": np.ascontiguousarray(
                    np.concatenate([vg.T, cvT_pad[b]], axis=1), np.float32),
                "c2h": np.ascontiguousarray(c2h_pad[b][None, :], np.float32),
            })
        nc2 = _get_program(U, rows=rows2, nt=ATm)
        res2 = run_bass_kernel_spmd(nc2, in2, core_ids=list(range(NCORES)))
        for core in range(NCORES):
            b = core // 2
            r0 = (core % 2) * ROWS
            fr = flag_rows[core]
            if not len(fr):
                continue
            jj = np.rint(res2.results[core]["idx"]).astype(np.int64)
            jflat = jj.T.reshape(-1)[:len(fr)]
            nn[b, r0 + fr] = first_pos[(U - 1) - jflat]

    batch_idx = np.broadcast_to(np.arange(B, dtype=np.int32)[:, None], nn.shape)
    outv = np.stack([batch_idx, nn], axis=-1).astype(np.int32)
    if _want_trace:
        return outv, (res1, in_maps, res2, in2, flag_rows)
    return outv
